# revision 1
# baseline (speedup 1.0000x reference)
"""Trainium2 Bass kernel for nn_AuxiliaryModel_57707180589353.

Tree-conv GNN-ish model:
  - per-leaf 1x1 conv (scalar -> C channels) + leaf node weight
  - per-unmatched-column 1x1 conv
  - 10 levels of pairwise tree merge: Conv1d(C,C,3,'same') + BN(eval) + ReLU,
    scaled by per-node weight; every level emits a [B, C, 1024] feature chunk
  - concat all chunks along length, max-pool adjacent pairs, flatten.

Sharding: data-parallel over batch B=256 across 8 cores (32 samples/core).
All parameters are tiny and replicated.

Device layout (per core): activations live as [128, 1024] SBUF tiles:
  partition p = 16*s + c  (s = sample-in-group 0..7, c = channel 0..15),
  free dim   = spatial in "split" order: col j holds position 2j (even half,
  cols 0..511) / position 2j+1 (odd half, cols 512..1023).
The split order makes the conv taps contiguous matmuls and the final
pair-max-pool a dense tensor_tensor(max) of the two halves.

Conv1d(C,C,3) is computed on the TensorEngine as 3 accumulated matmuls with
block-diagonal (8 groups x 16x16) weights; per-node 'same' zero padding is
realized by subtracting the spurious cross-node-boundary contributions with
negated-weight matmuls over strided column APs.
"""

import numpy as np
import ml_dtypes

B = 256
L = 1024
U = 256
C = 16
LEVELS = 10
EPS = 1e-5
N_CORES = 8
BPC = B // N_CORES          # 32 samples per core
SPG = 8                     # samples per matmul group (8*16 = 128 partitions)
GROUPS = BPC // SPG         # 4
T_OUT = (L + U + LEVELS * L) // 2   # 5760
OUT_COLS = C * T_OUT        # 92160

BF16 = ml_dtypes.bfloat16

_CACHE = {}


def _build_nc(reps=1):
    import concourse.bacc as bacc
    import concourse.tile as tile
    import concourse.mybir as mybir

    dt = mybir.dt
    f32 = dt.float32
    bf16 = dt.bfloat16
    Act = mybir.ActivationFunctionType
    Alu = mybir.AluOpType

    nc = bacc.Bacc("TRN2", target_bir_lowering=False, debug=False,
                   enable_asserts=False, num_devices=N_CORES)

    def din(name, shape, dtype=bf16):
        return nc.dram_tensor(name, list(shape), dtype, kind="ExternalInput").ap()

    x_d = din("x", [BPC, L + U], f32)
    lwB_d = din("lwB", [128, L])
    lbB_d = din("lbB", [128, L])
    uwB_d = din("uwB", [128, U])
    ubB_d = din("ubB", [128, U])
    Wc_d = din("Wc", [128, 128])
    W0_d = din("W0", [128, 128])
    W2_d = din("W2", [128, 128])
    nW0_d = din("nW0", [128, 128])
    nW2_d = din("nW2", [128, 128])
    R_d = din("R", [32, GROUPS * 128])
    sP_d = din("sP", [128, 1], f32)
    b2P_d = din("b2P", [128, 1], f32)
    nwB_d = din("nwB", [128, LEVELS * L])
    out_d = nc.dram_tensor("out", [BPC, OUT_COLS], f32, kind="ExternalOutput").ap()

    # [4, 8, 16, 5760] view of the output: (group, sample, channel, pooled col)
    out_v = out_d.rearrange("(g s) (c t) -> g s c t", g=GROUPS, c=C)

    with tile.TileContext(nc) as tc:
        with (tc.tile_pool(name="consts", bufs=1) as cpool,
              tc.tile_pool(name="work", bufs=2) as work,
              tc.tile_pool(name="curp", bufs=8) as curp,
              tc.tile_pool(name="poolp", bufs=6) as poolp,
              tc.tile_pool(name="psp", bufs=4, space="PSUM") as psp):
            # ---- constant loads ----
            lwB = cpool.tile_from(lwB_d)
            lbB = cpool.tile_from(lbB_d)
            R = cpool.tile_from(R_d)
            Wc = cpool.tile_from(Wc_d)
            W0 = cpool.tile_from(W0_d)
            W2 = cpool.tile_from(W2_d)
            nW0 = cpool.tile_from(nW0_d)
            nW2 = cpool.tile_from(nW2_d)
            sP = cpool.tile_from(sP_d)
            b2P = cpool.tile_from(b2P_d)
            uwB = cpool.tile_from(uwB_d)
            ubB = cpool.tile_from(ubB_d)
            nwB = cpool.tile_from(nwB_d)

            def mm(out, lhsT, rhs, start, stop):
                nc.tensor.matmul(out, lhsT, rhs, start=start, stop=stop,
                                 skip_group_check=True)

            for _rep in range(reps):
              # ---- input load ----
              x_sb = work.tile([BPC, L + U], f32, tag="xstage", bufs=2,
                               name=f"xs{_rep}")
              nc.sync.dma_start(out=x_sb, in_=x_d)
              xb = work.tile([BPC, L + U], bf16, tag="xb", bufs=2,
                             name=f"xb{_rep}")
              nc.vector.tensor_copy(out=xb, in_=x_sb)

              # ---- leaf stage: cur_{-1}[(s,c), j] = x[s,sig(j)]*lw'[sig(j),c] + lb' ----
              curs = []
              for g in range(GROUPS):
                  ps = psp.tile([128, L], f32, tag="ps", name=f"psleaf{g}")
                  mm(ps[:, 0:512], R[:, g * 128:(g + 1) * 128], xb[:, 0:L:2], True, True)
                  mm(ps[:, 512:1024], R[:, g * 128:(g + 1) * 128], xb[:, 1:L:2], True, True)
                  tmp = work.tile([128, L], bf16, tag="tmp", name=f"tmpleaf{g}")
                  nc.scalar.activation(out=tmp, in_=ps, func=Act.Copy, scale=1.0)
                  cur = curp.tile([128, L], bf16, tag="cur", name=f"curleaf{g}")
                  nc.vector.tensor_mul(out=cur, in0=tmp, in1=lwB)
                  nc.vector.tensor_add(out=cur, in0=cur, in1=lbB)
                  pooled = poolp.tile([128, 512], f32, tag="pooled",
                                      name=f"plleaf{g}")
                  nc.vector.tensor_tensor(out=pooled, in0=cur[:, 0:512],
                                          in1=cur[:, 512:1024], op=Alu.max)
                  nc.sync.dma_start(out=out_v[g, :, :, 0:512], in_=pooled)
                  curs.append(cur)

              # ---- unmatched columns: chunk at pooled offset 512, width 128 ----
              for g in range(GROUPS):
                  psu = psp.tile([128, U], f32, tag="ps", name=f"psunm{g}")
                  mm(psu[:, 0:128], R[:, g * 128:(g + 1) * 128], xb[:, L:L + U:2], True, True)
                  mm(psu[:, 128:256], R[:, g * 128:(g + 1) * 128], xb[:, L + 1:L + U:2], True, True)
                  tmpu = work.tile([128, U], bf16, tag="tmp", name=f"tmpunm{g}")
                  nc.scalar.activation(out=tmpu, in_=psu, func=Act.Copy, scale=1.0)
                  nc.vector.tensor_mul(out=tmpu, in0=tmpu, in1=uwB)
                  nc.vector.tensor_add(out=tmpu, in0=tmpu, in1=ubB)
                  pooledu = poolp.tile([128, U // 2], f32, tag="pooled",
                                       name=f"plunm{g}")
                  nc.vector.tensor_tensor(out=pooledu, in0=tmpu[:, 0:128],
                                          in1=tmpu[:, 128:256], op=Alu.max)
                  nc.sync.dma_start(out=out_v[g, :, :, 512:640], in_=pooledu)

              # ---- tree levels ----
              for k in range(LEVELS):
                  hl = 1 << k          # half node length in split-layout columns
                  n = 512 >> k         # number of nodes at this level
                  nwk = nwB[:, k * L:(k + 1) * L]
                  for g in range(GROUPS):
                      prev = curs[g]
                      ce, co = prev[:, 0:512], prev[:, 512:1024]
                      ps = psp.tile([128, L], f32, tag="ps", name=f"ps{k}_{g}")
                      pe, po = ps[:, 0:512], ps[:, 512:1024]
                      if k == 0:
                          # nodes are (leaf 2j, leaf 2j+1); kernel-3 'same' on len-2
                          mm(pe, Wc, ce, True, False)
                          mm(pe, W2, co, False, True)
                          mm(po, Wc, co, True, False)
                          mm(po, W0, ce, False, True)
                      else:
                          # even outputs: pos 2j reads 2j-1 (odd j-1), 2j, 2j+1
                          mm(pe, Wc, ce, True, False)
                          mm(pe[:, 1:512], W0, co[:, 0:511], False, False)
                          if n > 1:
                              # remove cross-node W0 term at node starts j*hl
                              mm(pe[:, hl:512:hl], nW0, co[:, hl - 1:511:hl],
                                 False, False)
                          mm(pe, W2, co, False, True)
                          # odd outputs: pos 2j+1 reads 2j, 2j+1, 2j+2 (even j+1)
                          mm(po, Wc, co, True, False)
                          mm(po[:, 0:511], W2, ce[:, 1:512], False, False)
                          if n > 1:
                              # remove cross-node W2 term at node ends j*hl-1
                              mm(po[:, hl - 1:511:hl], nW2, ce[:, hl:512:hl],
                                 False, False)
                          mm(po, W0, ce, False, True)
                      cur = curp.tile([128, L], bf16, tag="cur", name=f"cur{k}_{g}")
                      nc.scalar.activation(out=cur, in_=ps, func=Act.Relu,
                                           bias=b2P, scale=sP)
                      nc.vector.tensor_mul(out=cur, in0=cur, in1=nwk)
                      pooled = poolp.tile([128, 512], f32, tag="pooled",
                                          name=f"pl{k}_{g}")
                      nc.vector.tensor_tensor(out=pooled, in0=cur[:, 0:512],
                                              in1=cur[:, 512:1024], op=Alu.max)
                      off = 640 + 512 * k
                      nc.sync.dma_start(out=out_v[g, :, :, off:off + 512],
                                        in_=pooled)
                      curs[g] = cur

    nc.compile()
    return nc


def _split_cols(a):
    """Reorder the last axis from position order to split (even|odd) order."""
    return np.concatenate([a[..., 0::2], a[..., 1::2]], axis=-1)


def _host_consts(leaf_w, leaf_b, unm_w, unm_b, conv_w, conv_b,
                 bn_gamma, bn_beta, bn_mean, bn_var, leaf_nw, internal_nw):
    f32 = np.float32

    def rep_pc(v16):  # [16] -> [128, 1] (partition p = 16*s + c)
        return np.tile(np.asarray(v16, f32), SPG).reshape(128, 1)

    s = (bn_gamma / np.sqrt(bn_var + EPS)).astype(f32)
    b2 = ((conv_b - bn_mean) * s + bn_beta).astype(f32)

    lw = (leaf_w * leaf_nw[:, None]).astype(f32)      # [L, C]
    lb = (leaf_b * leaf_nw[:, None]).astype(f32)

    def bcast_cols(wLC):  # [Ncols, C] -> [128, Ncols] split order, bf16
        t = np.tile(wLC.T, (SPG, 1))                  # [128, Ncols]
        return _split_cols(t).astype(BF16)

    lwB = bcast_cols(lw)
    lbB = bcast_cols(lb)
    uwB = bcast_cols(np.asarray(unm_w, f32))
    ubB = bcast_cols(np.asarray(unm_b, f32))

    def blockdiag(w16):  # 16x16 block -> [128, 128] block-diagonal
        out = np.zeros((128, 128), f32)
        for g in range(SPG):
            out[g * C:(g + 1) * C, g * C:(g + 1) * C] = w16
        return out

    # lhsT[(g,ci),(g,co)] = conv_w[co, ci, k]
    Wk = [blockdiag(conv_w[:, :, k].T) for k in range(3)]
    Wc = Wk[1].astype(BF16)
    W0 = Wk[0].astype(BF16)
    W2 = Wk[2].astype(BF16)
    nW0 = (-Wk[0]).astype(BF16)
    nW2 = (-Wk[2]).astype(BF16)

    R = np.zeros((32, GROUPS * 128), f32)
    for g in range(GROUPS):
        for sl in range(SPG):
            R[g * SPG + sl, g * 128 + sl * C:g * 128 + (sl + 1) * C] = 1.0
    R = R.astype(BF16)

    # node-weight vectors per level, expanded to [128, 1024] in split order
    nwB = np.zeros((128, LEVELS * L), f32)
    off = 0
    for k in range(LEVELS):
        n = L >> (k + 1)
        w = np.asarray(internal_nw[off:off + n], f32)
        off += n
        expand = np.repeat(w, 1 << (k + 1))          # [1024] position order
        nwB[:, k * L:(k + 1) * L] = _split_cols(expand)[None, :]
    nwB = nwB.astype(BF16)

    return {
        "lwB": lwB, "lbB": lbB, "uwB": uwB, "ubB": ubB,
        "Wc": Wc, "W0": W0, "W2": W2, "nW0": nW0, "nW2": nW2,
        "R": R, "sP": rep_pc(s), "b2P": rep_pc(b2),
        "nwB": nwB,
    }


def kernel(x, leaf_w, leaf_b, unm_w, unm_b, conv_w, conv_b,
           bn_gamma, bn_beta, bn_mean, bn_var, leaf_nw, internal_nw):
    from concourse.bass_utils import run_bass_kernel_spmd

    if "nc" not in _CACHE:
        _CACHE["nc"] = _build_nc()
    nc = _CACHE["nc"]

    consts = _host_consts(
        np.asarray(leaf_w), np.asarray(leaf_b), np.asarray(unm_w),
        np.asarray(unm_b), np.asarray(conv_w), np.asarray(conv_b),
        np.asarray(bn_gamma), np.asarray(bn_beta), np.asarray(bn_mean),
        np.asarray(bn_var), np.asarray(leaf_nw), np.asarray(internal_nw))

    x = np.ascontiguousarray(np.asarray(x, np.float32))
    in_maps = []
    for c in range(N_CORES):
        m = dict(consts)
        m["x"] = np.ascontiguousarray(x[c * BPC:(c + 1) * BPC])
        in_maps.append(m)

    res = run_bass_kernel_spmd(nc, in_maps, core_ids=list(range(N_CORES)))
    out = np.concatenate([r["out"] for r in res.results], axis=0)
    return out.astype(np.float32)



# revision 5
# speedup vs baseline: 1.6293x; 1.6293x over previous
"""Trainium2 Bass kernel for nn_AuxiliaryModel_57707180589353.

Tree-conv model:
  - per-leaf 1x1 conv (scalar -> C channels) + leaf node weight
  - per-unmatched-column 1x1 conv
  - 10 levels of pairwise tree merge: Conv1d(C,C,3,'same') + BN(eval) + ReLU,
    scaled by per-node weight; every level emits a [B, C, 1024] feature chunk
  - concat all chunks along length, max-pool adjacent pairs, flatten.

Sharding: data-parallel over batch B=256 across 8 cores (32 samples/core).
All parameters are tiny and replicated.

Device layout (per core): activations live as [128, 1024] fp16 SBUF tiles:
  partition p = 16*s + c  (s = sample-in-group 0..7, c = channel 0..15),
  free dim   = spatial in "split" order: col j holds position 2j (even half,
  cols 0..511) / position 2j+1 (odd half, cols 512..1023).
The split order makes the conv taps contiguous matmuls and the final
pair-max-pool a dense tensor_tensor(max) of the two halves.

Conv1d(C,C,3) runs on the TensorEngine as accumulated matmuls with
block-diagonal (8 groups x 16x16) weights; BN scale is folded into the
weights host-side so the PSUM->SBUF pass is a plain bias+ReLU activation.
Per-node 'same' zero padding is realized by subtracting the spurious
cross-node-boundary contributions with negated-weight matmuls over strided
column APs.

Engine balance: TensorE does the convs densely (group-major, fused 1024-col
center-tap matmuls) so it stays at the warm p-state; ScalarE drains PSUM
(bias+ReLU); DVE applies node weights and the pair-max; pooled outputs
accumulate in per-group SBUF band buffers and stream out as fp16 (upcast on
host).
"""

import numpy as np

B = 256
L = 1024
U = 256
C = 16
LEVELS = 10
EPS = 1e-5
N_CORES = 8
BPC = B // N_CORES          # 32 samples per core
SPG = 8                     # samples per matmul group (8*16 = 128 partitions)
GROUPS = BPC // SPG         # 4
T_OUT = (L + U + LEVELS * L) // 2   # 5760
OUT_COLS = C * T_OUT        # 92160

FUSE_WC = False             # 1024-col matmuls exceed the ISA PSUM-dst limit

_CACHE = {}


def _build_nc():
    import concourse.bacc as bacc
    import concourse.tile as tile
    import concourse.mybir as mybir

    dt = mybir.dt
    f32 = dt.float32
    f16 = dt.float16
    Act = mybir.ActivationFunctionType
    Alu = mybir.AluOpType

    nc = bacc.Bacc("TRN2", target_bir_lowering=False, debug=False,
                   enable_asserts=False, num_devices=N_CORES)

    def din(name, shape, dtype=f16):
        return nc.dram_tensor(name, list(shape), dtype, kind="ExternalInput").ap()

    x_d = din("x", [BPC, L + U])
    lwB_d = din("lwB", [128, L])
    lbB_d = din("lbB", [128, L])
    uwB_d = din("uwB", [128, U])
    ubB_d = din("ubB", [128, U])
    Wc_d = din("Wc", [128, 128])
    W0_d = din("W0", [128, 128])
    W2_d = din("W2", [128, 128])
    nW0_d = din("nW0", [128, 128])
    nW2_d = din("nW2", [128, 128])
    R_d = din("R", [32, GROUPS * 128])
    b2P_d = din("b2P", [128, 1], f32)
    nwa_d = din("nwa", [128, 2 * L])            # levels 0-1
    nwb_d = din("nwb", [128, (LEVELS - 2) * L])  # levels 2-9
    out_d = nc.dram_tensor("out", [BPC, OUT_COLS], f16, kind="ExternalOutput").ap()

    # [4, 8, 16, 5760] view of the output: (group, sample, channel, pooled col)
    out_v = out_d.rearrange("(g s) (c t) -> g s c t", g=GROUPS, c=C)

    # pooled-column bands: (first level, n levels, out col offset)
    BANDS = [(0, 3, 640), (3, 3, 2176), (6, 3, 3712), (9, 1, 5248)]

    with tile.TileContext(nc) as tc:
        with (tc.tile_pool(name="consts", bufs=1) as cpool,
              tc.tile_pool(name="curp", bufs=8) as curp,
              tc.tile_pool(name="chp", bufs=8) as chp,
              tc.tile_pool(name="small", bufs=4) as smallp,
              tc.tile_pool(name="psp", bufs=4, space="PSUM") as psp):
            # ---- constant loads (order = DMA priority) ----
            R = cpool.tile_from(R_d)
            xb = cpool.tile_from(x_d)
            lwB = cpool.tile_from(lwB_d)
            lbB = cpool.tile_from(lbB_d)
            Wc = cpool.tile_from(Wc_d)
            W0 = cpool.tile_from(W0_d)
            W2 = cpool.tile_from(W2_d)
            nW0 = cpool.tile_from(nW0_d)
            nW2 = cpool.tile_from(nW2_d)
            b2P = cpool.tile_from(b2P_d)
            nwa = cpool.tile_from(nwa_d)
            uwB = cpool.tile_from(uwB_d)
            ubB = cpool.tile_from(ubB_d)
            nwb = cpool.tile_from(nwb_d)

            def mm(out, lhsT, rhs, start, stop):
                nc.tensor.matmul(out, lhsT, rhs, start=start, stop=stop,
                                 skip_group_check=True)

            # split-order view of the unmatched input: col j -> pos 2j / 2j+1
            rhs_unm = xb[:, L:L + U].rearrange("p (j t) -> p t j", t=2)

            # ---- leaf stage: cur[(s,c), j] = (x[s,sig(j)]*lw'+lb') ----
            ps_leaf = []
            for g in range(GROUPS):
                ps = psp.tile([128, L], f32, tag="ps", name=f"psleaf{g}")
                Rg = R[:, g * 128:(g + 1) * 128]
                mm(ps[:, 0:512], Rg, xb[:, 0:L:2], True, True)
                mm(ps[:, 512:1024], Rg, xb[:, 1:L:2], True, True)
                ps_leaf.append(ps)
            curs = []
            for g in range(GROUPS):
                cur = curp.tile([128, L], f16, tag="cur", name=f"lcur{g}")
                nc.scalar.activation(out=cur, in_=ps_leaf[g], func=Act.Copy,
                                     scale=1.0)
                nc.vector.tensor_mul(out=cur, in0=cur, in1=lwB)
                nc.vector.tensor_add(out=cur, in0=cur, in1=lbB)
                lch = chp.tile([128, 512], f16, tag="lch", bufs=4,
                               name=f"lch{g}")
                nc.vector.tensor_tensor(out=lch, in0=cur[:, 0:512],
                                        in1=cur[:, 512:1024], op=Alu.max)
                nc.sync.dma_start(out=out_v[g, :, :, 0:512], in_=lch)
                curs.append(cur)

            # ---- tree levels ----
            bands = {}
            for k0, nk, ooff in BANDS:
                for k in range(k0, k0 + nk):
                    hl = 1 << k          # half node length in split cols
                    n = 512 >> k         # number of nodes at this level
                    if k < 2:
                        nwk = nwa[:, k * L:(k + 1) * L]
                    else:
                        nwk = nwb[:, (k - 2) * L:(k - 1) * L]
                    for g in range(GROUPS):
                        prev = curs[g]
                        ce, co = prev[:, 0:512], prev[:, 512:1024]
                        ps = psp.tile([128, L], f32, tag="ps",
                                      name=f"ps{k}_{g}")
                        pe, po = ps[:, 0:512], ps[:, 512:1024]
                        # (dst, weights, src, is_first_write_of_its_region)
                        if k == 0:
                            # length-2 nodes: no shifted taps, no fixups
                            mms = [(pe, Wc, ce, True), (po, Wc, co, True),
                                   (pe, W2, co, False), (po, W0, ce, False)]
                        else:
                            mms = [(pe, Wc, ce, True), (po, Wc, co, True),
                                   (po, W0, ce, False),
                                   (pe[:, 1:512], W0, co[:, 0:511], False)]
                            if n > 1:
                                mms.append((pe[:, hl:512:hl], nW0,
                                            co[:, hl - 1:511:hl], False))
                            mms.append((pe, W2, co, False))
                            mms.append((po[:, 0:511], W2, ce[:, 1:512], False))
                            if n > 1:
                                mms.append((po[:, hl - 1:511:hl], nW2,
                                            ce[:, hl:512:hl], False))
                        for i, (o, w, r, first) in enumerate(mms):
                            mm(o, w, r, first, i == len(mms) - 1)
                        cur = curp.tile([128, L], f16, tag="cur",
                                        name=f"cur{k}_{g}")
                        nc.scalar.activation(out=cur, in_=ps, func=Act.Relu,
                                             bias=b2P, scale=1.0)
                        nc.vector.tensor_mul(out=cur, in0=cur, in1=nwk)
                        if k == k0:
                            bands[g] = chp.tile([128, nk * 512], f16,
                                                tag=f"band{nk}",
                                                name=f"band{k}_{g}")
                        bslice = bands[g][:, (k - k0) * 512:(k - k0 + 1) * 512]
                        nc.vector.tensor_tensor(out=bslice, in0=cur[:, 0:512],
                                                in1=cur[:, 512:1024],
                                                op=Alu.max)
                        if k == k0 + nk - 1:
                            nc.sync.dma_start(
                                out=out_v[g, :, :, ooff:ooff + nk * 512],
                                in_=bands[g])
                        curs[g] = cur

            # ---- unmatched columns (independent; emitted last) ----
            ps_unm = []
            for g in range(GROUPS):
                psu = psp.tile([128, U], f32, tag="ps", name=f"psunm{g}")
                mm(psu, R[:, g * 128:(g + 1) * 128], rhs_unm, True, True)
                ps_unm.append(psu)
            for g in range(GROUPS):
                ut = smallp.tile([128, U], f16, tag="ut", name=f"ut{g}")
                nc.scalar.activation(out=ut, in_=ps_unm[g], func=Act.Copy,
                                     scale=1.0)
                nc.vector.tensor_mul(out=ut, in0=ut, in1=uwB)
                nc.vector.tensor_add(out=ut, in0=ut, in1=ubB)
                uch = smallp.tile([128, U // 2], f16, tag="uch",
                                  name=f"uch{g}")
                nc.vector.tensor_tensor(out=uch, in0=ut[:, 0:128],
                                        in1=ut[:, 128:256], op=Alu.max)
                nc.sync.dma_start(out=out_v[g, :, :, 512:640], in_=uch)

    nc.compile()
    return nc


def _split_cols(a):
    """Reorder the last axis from position order to split (even|odd) order."""
    return np.concatenate([a[..., 0::2], a[..., 1::2]], axis=-1)


def _host_consts(leaf_w, leaf_b, unm_w, unm_b, conv_w, conv_b,
                 bn_gamma, bn_beta, bn_mean, bn_var, leaf_nw, internal_nw):
    f32 = np.float32
    f16 = np.float16

    s = (bn_gamma / np.sqrt(bn_var + EPS)).astype(f32)          # [C]
    b2 = ((conv_b - bn_mean) * s + bn_beta).astype(f32)

    lw = (leaf_w * leaf_nw[:, None]).astype(f32)                # [L, C]
    lb = (leaf_b * leaf_nw[:, None]).astype(f32)

    def rep_pc(v16):  # [16] -> [128, 1] (partition p = 16*s + c)
        return np.tile(np.asarray(v16, f32), SPG).reshape(128, 1)

    def bcast_cols(wLC):  # [Ncols, C] -> [128, Ncols] split order
        t = np.tile(wLC.T, (SPG, 1))                            # [128, Ncols]
        return _split_cols(t).astype(f16)

    lwB = bcast_cols(lw)
    lbB = bcast_cols(lb)
    uwB = bcast_cols(np.asarray(unm_w, f32))
    ubB = bcast_cols(np.asarray(unm_b, f32))

    def blockdiag(w16):  # 16x16 block -> [128, 128] block-diagonal
        out = np.zeros((128, 128), f32)
        for g in range(SPG):
            out[g * C:(g + 1) * C, g * C:(g + 1) * C] = w16
        return out

    # BN scale folded into the conv weights:
    # lhsT[(g,ci),(g,co)] = conv_w[co, ci, k] * s[co]
    Wk = [blockdiag((conv_w[:, :, k] * s[:, None]).T) for k in range(3)]
    Wc = Wk[1].astype(f16)
    W0 = Wk[0].astype(f16)
    W2 = Wk[2].astype(f16)
    nW0 = (-Wk[0]).astype(f16)
    nW2 = (-Wk[2]).astype(f16)

    R = np.zeros((32, GROUPS * 128), f32)
    for g in range(GROUPS):
        for sl in range(SPG):
            R[g * SPG + sl, g * 128 + sl * C:g * 128 + (sl + 1) * C] = 1.0
    R = R.astype(f16)

    # node-weight vectors per level, expanded to [128, 1024] in split order
    nwB = np.zeros((128, LEVELS * L), f32)
    off = 0
    for k in range(LEVELS):
        n = L >> (k + 1)
        w = np.asarray(internal_nw[off:off + n], f32)
        off += n
        expand = np.repeat(w, 1 << (k + 1))          # [1024] position order
        nwB[:, k * L:(k + 1) * L] = _split_cols(expand)[None, :]
    nwB = nwB.astype(f16)

    return {
        "lwB": lwB, "lbB": lbB, "uwB": uwB, "ubB": ubB,
        "Wc": Wc, "W0": W0, "W2": W2, "nW0": nW0, "nW2": nW2,
        "R": R, "b2P": rep_pc(b2),
        "nwa": np.ascontiguousarray(nwB[:, 0:2 * L]),
        "nwb": np.ascontiguousarray(nwB[:, 2 * L:]),
    }


def _make_in_maps(inputs):
    consts = _host_consts(
        np.asarray(inputs["leaf_w"]), np.asarray(inputs["leaf_b"]),
        np.asarray(inputs["unm_w"]), np.asarray(inputs["unm_b"]),
        np.asarray(inputs["conv_w"]), np.asarray(inputs["conv_b"]),
        np.asarray(inputs["bn_gamma"]), np.asarray(inputs["bn_beta"]),
        np.asarray(inputs["bn_mean"]), np.asarray(inputs["bn_var"]),
        np.asarray(inputs["leaf_nw"]), np.asarray(inputs["internal_nw"]))
    x = np.asarray(inputs["x"], np.float32).astype(np.float16)
    in_maps = []
    for c in range(N_CORES):
        m = dict(consts)
        m["x"] = np.ascontiguousarray(x[c * BPC:(c + 1) * BPC])
        in_maps.append(m)
    return in_maps


def kernel(x, leaf_w, leaf_b, unm_w, unm_b, conv_w, conv_b,
           bn_gamma, bn_beta, bn_mean, bn_var, leaf_nw, internal_nw):
    from concourse.bass_utils import run_bass_kernel_spmd

    if "nc" not in _CACHE:
        _CACHE["nc"] = _build_nc()
    nc = _CACHE["nc"]

    in_maps = _make_in_maps({
        "x": x, "leaf_w": leaf_w, "leaf_b": leaf_b, "unm_w": unm_w,
        "unm_b": unm_b, "conv_w": conv_w, "conv_b": conv_b,
        "bn_gamma": bn_gamma, "bn_beta": bn_beta, "bn_mean": bn_mean,
        "bn_var": bn_var, "leaf_nw": leaf_nw, "internal_nw": internal_nw})

    res = run_bass_kernel_spmd(nc, in_maps, core_ids=list(range(N_CORES)))
    out = np.concatenate([r["out"] for r in res.results], axis=0)
    return out.astype(np.float32)
